# revision 24
# baseline (speedup 1.0000x reference)
"""Trainium2 Bass kernel for nn_DGraphAttention (gnn_message_passing).

Math (reference):
    x = hidden_states.reshape(N, H)
    q/k/v = x @ W{q,k,v}.T + b
    src, tgt = sort(edges_src), sort(edges_tgt)        # [E] each
    scores = softmax((q[tgt] @ k[src].T) / sqrt(HEAD), axis=0)   # over tgt axis
    v[tgt] = scores @ v[src]
    return v.reshape(B, S, H)

Sharding (8 cores):
  - tgt rows of the E x E score matrix split 1024/core
  - v_src sharded 1024 rows/core (bf16) + AllGather (replaces the 8x-redundant
    per-core recompute)
  - v_own covers only this core's 3072 NON-tgt node rows (tgt rows of the
    linear output are overwritten by the attention scatter anyway)
  - k projection folded into W2 = Wq^T Wk on the host; per-tgt bias term
    becomes the multiplicative factor g[i] applied on the fly during the
    colsum reduce and once on the final output
  - softmax normalizer: one AllReduce of a [128, 64] f32 colsum at the end of
    the scores loop, hidden under the v_own phase; the collective readback is
    issued from the gpsimd queue so it cannot stall the sync-engine DMA queue

Precision: the two big E*E*H matmuls (scores, scores@v) run in fp8 e4m3 with
DoubleRow (2 MACs/cell/cycle); exp-scores stay resident in SBUF (8MB fp8).
exp exponent is biased by -2 and v rows are scaled by 2048/colsum to center
everything in fp8 range (the scales cancel exactly). Validated numerically:
global rel_l2 ~ 8e-4 vs the f32 reference.
"""

import os
import sys

sys.path.insert(0, "/opt/trn_rl_repo")

import numpy as np
from contextlib import ExitStack

import concourse.bass as bass
import concourse.bacc as bacc
import concourse.mybir as mybir
from concourse.tile import TileContext
from concourse.tile_rust import add_dep_helper
from concourse.bass_utils import run_bass_kernel_spmd

F32 = mybir.dt.float32
F32R = mybir.dt.float32r
BF16 = mybir.dt.bfloat16
F8 = mybir.dt.float8e4
AF = mybir.ActivationFunctionType
ALU = mybir.AluOpType
DR = mybir.MatmulPerfMode.DoubleRow

# problem constants
N_CORES = 8
B, S, H, NH = 4, 8192, 512, 8
HEAD = H // NH          # 64
N = B * S               # 32768
E = 8192
P = 128
FT = H // P             # 4 feature subtiles

N_TGT = E // N_CORES    # 1024 tgt score rows per core
N_SRCO = E // N_CORES   # 1024 src rows per core (v_src shard)
N_OWN = (N - E) // N_CORES  # 3072 non-tgt node rows per core
JT = E // P             # 64 src row tiles
JBLK = 512              # src rows per xs DMA block
NJB = E // JBLK         # 16
EXP_BIAS = -2.0         # exp exponent bias (cancels in normalization)
VSCALE = 2048.0         # v/colsum prescale into fp8 range (cancels exactly)

LAST_RESULT = None
_PROGRAM = None


def build_program():
    nc = bacc.Bacc(num_devices=N_CORES)

    # ---- DRAM parameters ----
    xs8 = nc.declare_dram_parameter("xs8", [H, E], F8, isOutput=False)
    xso = nc.declare_dram_parameter("xso", [H, N_SRCO], BF16, isOutput=False)
    w2b = nc.declare_dram_parameter("w2b", [H, H], BF16, isOutput=False)
    xtgb = nc.declare_dram_parameter("xtgb", [H, N_TGT], BF16, isOutput=False)
    b2_t = nc.declare_dram_parameter("b2_t", [P, FT], F32, isOutput=False)
    g_bc = nc.declare_dram_parameter("g_bc", [P, N_TGT], F32, isOutput=False)
    wvbT = nc.declare_dram_parameter("wvbT", [H, H], BF16, isOutput=False)
    bv_row = nc.declare_dram_parameter("bv_row", [1, H], BF16, isOutput=False)
    xT_own = nc.declare_dram_parameter("xT_own", [H, N_OWN], BF16, isOutput=False)
    v_own = nc.declare_dram_parameter("v_own", [N_OWN, H], BF16, isOutput=True)
    outT_tgt = nc.declare_dram_parameter("outT_tgt", [H, N_TGT], F32, isOutput=True)

    # internal DRAM for collectives
    cc_vin = nc.dram_tensor("cc_vin", [N_SRCO, H], BF16)
    cc_vout = nc.dram_tensor("cc_vout", [E, H], BF16, addr_space="Shared")
    cc_in_a = nc.dram_tensor("cc_in_a", [P, JT // 2], F32)
    cc_out_a = nc.dram_tensor("cc_out_a", [P, JT // 2], F32, addr_space="Shared")
    cc_in_b = nc.dram_tensor("cc_in_b", [P, JT // 2], F32)
    cc_out_b = nc.dram_tensor("cc_out_b", [P, JT // 2], F32, addr_space="Shared")

    rg = [list(range(N_CORES))]

    with TileContext(nc) as tc, ExitStack() as ctx:
        persist = ctx.enter_context(tc.tile_pool(name="persist", bufs=1))

        # persistent SBUF; DMAs in phase-priority order, split <=128KB per
        # queue (per-queue DMA bandwidth is only ~31 GB/s); small consts first
        b2_sb = persist.tile([P, FT], F32)
        nc.sync.dma_start(b2_sb[:], b2_t[:])
        bvr_sb = persist.tile([1, H], BF16)
        nc.sync.dma_start(bvr_sb[:], bv_row[:])
        gbc_sb = persist.tile([P, N_TGT], F32)
        nc.sync.dma_start(gbc_sb[:], g_bc[:])
        wvb_sb = persist.tile([P, FT, H], BF16)
        wvb_d = wvbT.rearrange("(ft p) f -> ft p f", p=P)
        for ft in range(FT):
            nc.sync.dma_start(wvb_sb[:, ft, :], wvb_d[ft])
        xso_sb = persist.tile([P, FT, N_SRCO], BF16)
        xso_d = xso.rearrange("(ft p) j -> ft p j", p=P)
        for ft in range(FT):
            for jh in range(2):
                nc.sync.dma_start(
                    xso_sb[:, ft, jh * 512:(jh + 1) * 512],
                    xso_d[ft][:, jh * 512:(jh + 1) * 512])
        w2b_sb = persist.tile([P, FT, H], BF16)
        w2b_d = w2b.rearrange("(ft p) f -> ft p f", p=P)
        for ft in range(FT):
            nc.sync.dma_start(w2b_sb[:, ft, :], w2b_d[ft])
        xtg_sb = persist.tile([P, FT, N_TGT], BF16)
        xtg_d = xtgb.rearrange("(ft p) i -> ft p i", p=P)
        for ft in range(FT):
            for ih in range(2):
                nc.sync.dma_start(
                    xtg_sb[:, ft, ih * 512:(ih + 1) * 512],
                    xtg_d[ft][:, ih * 512:(ih + 1) * 512])

        q_sb = persist.tile([P, FT, N_TGT], F8)
        e_sb = persist.tile([P, JT, N_TGT], F8)         # 8 MB resident
        colsum_sb = persist.tile([P, JT], F32)
        csg_sb = persist.tile([P, JT], F32)
        recip2_sb = persist.tile([P, JT], F32)
        ebias_sb = persist.tile([P, 1], F32)
        nc.vector.memset(ebias_sb[:], EXP_BIAS)
        ones_sb = persist.tile([1, P], BF16)
        nc.vector.memset(ones_sb[:], 1.0)

        # ---- phase V: v_src own shard [1024, H] bf16 -> AllGather ----
        # bias folded in as a ones-row matmul; psum evacuated on ScalarE
        vstores = []
        with (
            tc.tile_pool(name="psvq", bufs=4, space="PSUM") as psvq,
            tc.tile_pool(name="vt", bufs=3) as vtp,
        ):
            cc_vin_t = cc_vin.rearrange("(j4 p) f -> j4 p f", p=P)
            for j4 in range(N_SRCO // P):
                pv = psvq.tile([P, H], F32)
                for fs in range(FT):
                    nc.tensor.matmul(
                        pv[:],
                        xso_sb[:, fs, j4 * P:(j4 + 1) * P],
                        wvb_sb[:, fs, :],
                        start=(fs == 0), stop=False,
                    )
                nc.tensor.matmul(pv[:], ones_sb[:], bvr_sb[:],
                                 start=False, stop=True)
                vt = vtp.tile([P, H], BF16)
                nc.scalar.copy(vt[:], pv[:])
                d = nc.sync.dma_start(cc_vin_t[j4], vt[:])
                vstores.append(d)
            ag = nc.gpsimd.collective_compute(
                "AllGather", ALU.bypass, replica_groups=rg,
                ins=[cc_vin[:]], outs=[cc_vout[:]],
            )
            for d in vstores:
                add_dep_helper(ag.ins, d.ins, sync=True,
                               reason="v_src stores before allgather")

            # ---- phase Q: q_ext^T = W2^T x_tgt + b2, cast to fp8 ----
            for ft in range(FT):
                for ic in range(2):
                    pq = psvq.tile([P, 512], F32)
                    for fs in range(FT):
                        nc.tensor.matmul(
                            pq[:],
                            w2b_sb[:, fs, ft * P:(ft + 1) * P],
                            xtg_sb[:, fs, ic * 512:(ic + 1) * 512],
                            start=(fs == 0), stop=(fs == FT - 1),
                        )
                    nc.scalar.activation(
                        q_sb[:, ft, ic * 512:(ic + 1) * 512], pq[:],
                        AF.Identity, bias=b2_sb[:, ft:ft + 1],
                    )

        # ---- A/B loop: fp8 DoubleRow scores, exp (fp8), g-weighted colsum ----
        xop = ctx.enter_context(tc.tile_pool(name="xo", bufs=3))
        xo_tiles = []
        oc_n = N_OWN // 512  # 6 chunks for phase E
        xs8_d = xs8.rearrange("(ft p) j -> ft p j", p=P)
        xo_d = xT_own.rearrange("(ft p) o -> ft p o", p=P)

        with (
            tc.tile_pool(name="xs", bufs=3) as xsp,
            tc.tile_pool(name="scr", bufs=2) as scrp,
            tc.tile_pool(name="pss", bufs=2, space="PSUM") as pss,
        ):
            for jb in range(NJB):
                xs = xsp.tile([P, FT, JBLK], F8, tag="xs")
                for ft in range(FT):
                    nc.sync.dma_start(
                        xs[:, ft, :],
                        xs8_d[ft][:, jb * JBLK:(jb + 1) * JBLK])
                for jp in range(JBLK // (2 * P)):   # jt pairs
                    jt0 = jb * (JBLK // P) + 2 * jp
                    ps = pss.tile([P, 2, N_TGT], F32)   # 4 PSUM banks
                    for h in range(2):
                        j4 = 2 * jp + h
                        for ic in range(2):
                            for k in range(2):
                                nc.tensor.matmul(
                                    ps[:, h, ic * 512:(ic + 1) * 512],
                                    xs[:, 2 * k:2 * k + 2,
                                       j4 * P:(j4 + 1) * P],
                                    q_sb[:, 2 * k:2 * k + 2,
                                         ic * 512:(ic + 1) * 512],
                                    start=(k == 0), stop=(k == 1),
                                    perf_mode=DR,
                                )
                    nc.scalar.activation(
                        e_sb[:, jt0:jt0 + 2, :], ps[:],
                        AF.Exp, scale=float(1.0 / np.sqrt(HEAD)),
                        bias=ebias_sb[:],
                    )
                    for h in range(2):
                        jt = jt0 + h
                        scr = scrp.tile([P, N_TGT], BF16, tag="scr")
                        nc.vector.scalar_tensor_tensor(
                            scr[:], e_sb[:, jt, :], 1.0, gbc_sb[:],
                            op0=ALU.bypass, op1=ALU.mult,
                            accum_out=colsum_sb[:, jt:jt + 1],
                        )
                # first-half colsum AllReduce, entirely on the gpsimd queue so
                # no engine instruction stream is ever blocked behind it
                if jb == NJB // 2 - 1:
                    d1a = nc.gpsimd.dma_start(cc_in_a[:],
                                              colsum_sb[:, :JT // 2])
                    ara = nc.gpsimd.collective_compute(
                        "AllReduce", ALU.add, replica_groups=rg,
                        ins=[cc_in_a[:]], outs=[cc_out_a[:]],
                    )
                    add_dep_helper(ara.ins, d1a.ins, sync=True,
                                   reason="colsum_a store before allreduce")
                    d2a = nc.gpsimd.dma_start(csg_sb[:, :JT // 2],
                                              cc_out_a[:])
                    add_dep_helper(d2a.ins, ara.ins, sync=True,
                                   reason="allreduce_a before readback")
                # prefetch first phase-E x chunks under the A/B loop
                if jb in (10, 12, 14):
                    oc = (jb - 10) // 2
                    xo = xop.tile([P, FT, 512], BF16, tag="xo")
                    for ft in range(FT):
                        nc.sync.dma_start(
                            xo[:, ft, :],
                            xo_d[ft][:, oc * 512:(oc + 1) * 512])
                    xo_tiles.append(xo)

        # ---- second-half colsum AllReduce (gpsimd queue) + both recips ----
        d1b = nc.gpsimd.dma_start(cc_in_b[:], colsum_sb[:, JT // 2:])
        arb = nc.gpsimd.collective_compute(
            "AllReduce", ALU.add, replica_groups=rg,
            ins=[cc_in_b[:]], outs=[cc_out_b[:]],
        )
        add_dep_helper(arb.ins, d1b.ins, sync=True,
                       reason="colsum_b store before allreduce")
        d2b = nc.gpsimd.dma_start(csg_sb[:, JT // 2:], cc_out_b[:])
        add_dep_helper(d2b.ins, arb.ins, sync=True,
                       reason="allreduce_b before readback")
        nc.vector.reciprocal(recip2_sb[:, :JT // 2], csg_sb[:, :JT // 2])
        nc.vector.tensor_scalar_mul(
            recip2_sb[:, :JT // 2], recip2_sb[:, :JT // 2], VSCALE)
        nc.vector.reciprocal(recip2_sb[:, JT // 2:], csg_sb[:, JT // 2:])
        nc.vector.tensor_scalar_mul(
            recip2_sb[:, JT // 2:], recip2_sb[:, JT // 2:], VSCALE)

        # ---- phase E: v_own = x_own @ Wv.T + bv (bf16; overlaps AllReduce) ----
        with (
            tc.tile_pool(name="vo", bufs=3) as vop,
            tc.tile_pool(name="pse", bufs=2, space="PSUM") as pse,
        ):
            v_own_t = v_own.rearrange("(ot p) f -> ot p f", p=P)
            for oc in range(oc_n):
                if oc < len(xo_tiles):
                    xo = xo_tiles[oc]
                else:
                    xo = xop.tile([P, FT, 512], BF16, tag="xo")
                    for ft in range(FT):
                        nc.sync.dma_start(
                            xo[:, ft, :],
                            xo_d[ft][:, oc * 512:(oc + 1) * 512])
                for o4 in range(4):
                    pe_ = pse.tile([P, H], F32)
                    for fs in range(FT):
                        nc.tensor.matmul(
                            pe_[:],
                            xo[:, fs, o4 * P:(o4 + 1) * P],
                            wvb_sb[:, fs, :],
                            start=(fs == 0), stop=False,
                        )
                    nc.tensor.matmul(pe_[:], ones_sb[:], bvr_sb[:],
                                     start=False, stop=True)
                    vo = vop.tile([P, H], BF16)
                    nc.scalar.copy(vo[:], pe_[:])
                    nc.sync.dma_start(v_own_t[oc * 4 + o4], vo[:])

        # ---- phase C/D: out^T = e^T-matmul with (v*2048/colsum) in fp8 DR ----
        with (
            tc.tile_pool(name="cvb", bufs=10) as cvb,
            tc.tile_pool(name="cv8", bufs=3) as cv8,
            tc.tile_pool(name="co", bufs=2) as cop,
            tc.tile_pool(name="psc", bufs=1, space="PSUM") as pscp,
        ):
            psc_f = [pscp.tile([P, N_TGT], F32, tag=f"psc{f}", name=f"psc{f}")
                     for f in range(FT)]
            cc_vout_t = cc_vout.rearrange("(jt p) f -> jt p f", p=P)
            for t in range(JT // 2):
                v2 = cv8.tile([P, 2, H], F8, tag="v2")
                for k in range(2):
                    jt = 2 * t + k
                    vt = cvb.tile([P, H], BF16, tag="vt")
                    dv = nc.sync.dma_start(vt[:], cc_vout_t[jt])
                    add_dep_helper(dv.ins, ag.ins, sync=True,
                                   reason="allgather before v reload")
                    nc.scalar.mul(v2[:, k, :], vt[:],
                                  recip2_sb[:, jt:jt + 1])
                for ft in range(FT):
                    for ic in range(2):
                        nc.tensor.matmul(
                            psc_f[ft][:, ic * 512:(ic + 1) * 512],
                            v2[:, :, ft * P:(ft + 1) * P],
                            e_sb[:, 2 * t:2 * t + 2, ic * 512:(ic + 1) * 512],
                            start=(t == 0), stop=(t == JT // 2 - 1),
                            perf_mode=DR,
                        )
            for ft in range(FT):
                ot = cop.tile([P, N_TGT], F32)
                nc.vector.scalar_tensor_tensor(
                    ot[:], psc_f[ft][:], float(1.0 / VSCALE), gbc_sb[:],
                    op0=ALU.mult, op1=ALU.mult,
                )
                nc.sync.dma_start(outT_tgt[ft * P:(ft + 1) * P, :], ot[:])

    nc.compile()
    return nc


def _get_program():
    global _PROGRAM
    if _PROGRAM is None:
        _PROGRAM = build_program()
    return _PROGRAM


def make_in_maps(hidden_states, Wq, bq, Wk, bk, Wv, bv, edges_src, edges_tgt):
    """Host-side sharding: sort indices, gather rows, fold weights, cast."""
    import ml_dtypes
    BF = ml_dtypes.bfloat16
    F8NP = ml_dtypes.float8_e4m3

    x = np.ascontiguousarray(
        np.asarray(hidden_states, dtype=np.float32).reshape(N, H))
    src = np.sort(np.asarray(edges_src).astype(np.int64))
    tgt = np.sort(np.asarray(edges_tgt).astype(np.int64))
    mask = np.ones(N, bool)
    mask[tgt] = False
    nontgt = np.nonzero(mask)[0]
    xT = np.ascontiguousarray(x.T)                      # [H, N]

    # weight folding (f64 for exactness)
    Wq64 = np.asarray(Wq, np.float64)
    Wk64 = np.asarray(Wk, np.float64)
    bq64 = np.asarray(bq, np.float64)
    bk64 = np.asarray(bk, np.float64)
    W2 = (Wq64.T @ Wk64).astype(np.float32)
    b2 = (bq64 @ Wk64).astype(np.float32)
    wc = (Wq64.T @ bk64).astype(np.float32)
    beta = float(bq64 @ bk64)

    xT_src = xT[:, src]
    xs8 = np.ascontiguousarray(xT_src.astype(F8NP))
    xso_all = np.ascontiguousarray(xT_src.astype(BF))
    w2b = np.ascontiguousarray(W2.astype(BF))
    b2_t = np.ascontiguousarray(b2.reshape(FT, P).T)
    wvT = np.ascontiguousarray(np.asarray(Wv, np.float32).T)
    wvbT = np.ascontiguousarray(wvT.astype(BF))
    bv_row = np.ascontiguousarray(
        np.asarray(bv, np.float32)[None, :].astype(BF))

    in_maps = []
    for c in range(N_CORES):
        tgt_c = tgt[c * N_TGT:(c + 1) * N_TGT]
        x_tgt_c = np.ascontiguousarray(xT[:, tgt_c])
        g = np.exp((x_tgt_c.T.astype(np.float64) @ wc.astype(np.float64)
                    + beta) / np.sqrt(HEAD)).astype(np.float32)
        in_maps.append({
            "xs8": xs8,
            "xso": np.ascontiguousarray(
                xso_all[:, c * N_SRCO:(c + 1) * N_SRCO]),
            "w2b": w2b,
            "xtgb": np.ascontiguousarray(x_tgt_c.astype(BF)),
            "b2_t": b2_t,
            "g_bc": np.ascontiguousarray(np.tile(g[None, :], (P, 1))),
            "wvbT": wvbT,
            "bv_row": bv_row,
            "xT_own": np.ascontiguousarray(
                xT[:, nontgt[c * N_OWN:(c + 1) * N_OWN]].astype(BF)),
        })
    return in_maps, tgt, nontgt


def assemble_output(results, tgt, nontgt):
    v = np.empty((N, H), np.float32)
    for c in range(N_CORES):
        v[nontgt[c * N_OWN:(c + 1) * N_OWN]] = \
            results[c]["v_own"].astype(np.float32)
        v[tgt[c * N_TGT:(c + 1) * N_TGT]] = results[c]["outT_tgt"].T
    return v.reshape(B, S, H)


def kernel(hidden_states, Wq, bq, Wk, bk, Wv, bv, edges_src, edges_tgt):
    global LAST_RESULT
    in_maps, tgt, nontgt = make_in_maps(
        hidden_states, Wq, bq, Wk, bk, Wv, bv, edges_src, edges_tgt)
    nc = _get_program()
    res = run_bass_kernel_spmd(nc, in_maps, list(range(N_CORES)))
    LAST_RESULT = res
    return assemble_output(res.results, tgt, nontgt)


# revision 35
# speedup vs baseline: 1.0475x; 1.0475x over previous
"""Trainium2 Bass kernel for nn_DGraphAttention (gnn_message_passing).

Math (reference):
    x = hidden_states.reshape(N, H)
    q/k/v = x @ W{q,k,v}.T + b
    src, tgt = sort(edges_src), sort(edges_tgt)        # [E] each
    scores = softmax((q[tgt] @ k[src].T) / sqrt(HEAD), axis=0)   # over tgt axis
    v[tgt] = scores @ v[src]
    return v.reshape(B, S, H)

Sharding (8 cores):
  - tgt rows of the E x E score matrix split 1024/core
  - v_src sharded 1024 rows/core (bf16) + AllGather (replaces the 8x-redundant
    per-core recompute)
  - v_own covers only this core's 3072 NON-tgt node rows (tgt rows of the
    linear output are overwritten by the attention scatter anyway)
  - k projection folded into W2 = Wq^T Wk on the host; per-tgt bias term
    becomes the multiplicative factor g[i] applied on the fly during the
    colsum reduce and once on the final output
  - softmax normalizer: one AllReduce of a [128, 64] f32 colsum at the end of
    the scores loop, hidden under the v_own phase; the collective readback is
    issued from the gpsimd queue so it cannot stall the sync-engine DMA queue

Precision: the two big E*E*H matmuls (scores, scores@v) run in fp8 e4m3 with
DoubleRow (2 MACs/cell/cycle); exp-scores stay resident in SBUF (8MB fp8).
exp exponent is biased by -2 and v rows are scaled by 2048/colsum to center
everything in fp8 range (the scales cancel exactly). Validated numerically:
global rel_l2 ~ 8e-4 vs the f32 reference.
"""

import os
import sys

sys.path.insert(0, "/opt/trn_rl_repo")

import numpy as np
from contextlib import ExitStack

import concourse.bass as bass
import concourse.bacc as bacc
import concourse.mybir as mybir
from concourse.tile import TileContext
from concourse.tile_rust import add_dep_helper
from concourse.bass_utils import run_bass_kernel_spmd

F32 = mybir.dt.float32
F32R = mybir.dt.float32r
BF16 = mybir.dt.bfloat16
F8 = mybir.dt.float8e4
AF = mybir.ActivationFunctionType
ALU = mybir.AluOpType
DR = mybir.MatmulPerfMode.DoubleRow

# problem constants
N_CORES = 8
B, S, H, NH = 4, 8192, 512, 8
HEAD = H // NH          # 64
N = B * S               # 32768
E = 8192
P = 128
FT = H // P             # 4 feature subtiles

N_TGT = E // N_CORES    # 1024 tgt score rows per core
N_SRCO = E // N_CORES   # 1024 src rows per core (v_src shard)
N_OWN = (N - E) // N_CORES  # 3072 non-tgt node rows per core
JT = E // P             # 64 src row tiles
JBLK = 512              # src rows per xs DMA block
NJB = E // JBLK         # 16
EXP_BIAS = -2.0         # exp exponent bias (cancels in normalization)
VSCALE = 2048.0         # v/colsum prescale into fp8 range (cancels exactly)

LAST_RESULT = None
_PROGRAM = None


def build_program():
    nc = bacc.Bacc(num_devices=N_CORES)

    # ---- DRAM parameters ----
    xs8 = nc.declare_dram_parameter("xs8", [H, E], F8, isOutput=False)
    xso = nc.declare_dram_parameter("xso", [H, N_SRCO], BF16, isOutput=False)
    w2b = nc.declare_dram_parameter("w2b", [H, H], BF16, isOutput=False)
    xtgb = nc.declare_dram_parameter("xtgb", [H, N_TGT], BF16, isOutput=False)
    b2_t = nc.declare_dram_parameter("b2_t", [P, FT], F32, isOutput=False)
    g_bc = nc.declare_dram_parameter("g_bc", [P, N_TGT], F32, isOutput=False)
    wvbT = nc.declare_dram_parameter("wvbT", [H, H], BF16, isOutput=False)
    bv_bc = nc.declare_dram_parameter("bv_bc", [P, H], F32, isOutput=False)
    xT_own = nc.declare_dram_parameter("xT_own", [H, N_OWN], BF16, isOutput=False)
    v_own = nc.declare_dram_parameter("v_own", [N_OWN, H], BF16, isOutput=True)
    outT_tgt = nc.declare_dram_parameter("outT_tgt", [H, N_TGT], F32, isOutput=True)

    # internal DRAM for collectives
    cc_vin = nc.dram_tensor("cc_vin", [N_SRCO, H], BF16)
    cc_vout = nc.dram_tensor("cc_vout", [E, H], BF16, addr_space="Shared")
    cc_in_a = nc.dram_tensor("cc_in_a", [P, JT // 2], F32)
    cc_out_a = nc.dram_tensor("cc_out_a", [P, JT // 2], F32, addr_space="Shared")
    cc_in_b = nc.dram_tensor("cc_in_b", [P, JT // 2], F32)
    cc_out_b = nc.dram_tensor("cc_out_b", [P, JT // 2], F32, addr_space="Shared")

    rg = [list(range(N_CORES))]

    with TileContext(nc) as tc, ExitStack() as ctx:
        persist = ctx.enter_context(tc.tile_pool(name="persist", bufs=1))

        # persistent SBUF; DMAs in phase-priority order, split <=128KB per
        # queue (per-queue DMA bandwidth is only ~31 GB/s); small consts first
        b2_sb = persist.tile([P, FT], F32)
        nc.sync.dma_start(b2_sb[:], b2_t[:])
        bvb_sb = persist.tile([P, H], F32)
        nc.sync.dma_start(bvb_sb[:], bv_bc[:])
        gbc_sb = persist.tile([P, N_TGT], F32)
        nc.sync.dma_start(gbc_sb[:], g_bc[:])
        wvb_sb = persist.tile([P, FT, H], BF16)
        wvb_d = wvbT.rearrange("(ft p) f -> ft p f", p=P)
        for ft in range(FT):
            nc.sync.dma_start(wvb_sb[:, ft, :], wvb_d[ft])
        xso_sb = persist.tile([P, FT, N_SRCO], BF16)
        xso_d = xso.rearrange("(ft p) j -> ft p j", p=P)
        for ft in range(FT):
            for jh in range(2):
                nc.sync.dma_start(
                    xso_sb[:, ft, jh * 512:(jh + 1) * 512],
                    xso_d[ft][:, jh * 512:(jh + 1) * 512])
        w2b_sb = persist.tile([P, FT, H], BF16)
        w2b_d = w2b.rearrange("(ft p) f -> ft p f", p=P)
        for ft in range(FT):
            nc.sync.dma_start(w2b_sb[:, ft, :], w2b_d[ft])
        xtg_sb = persist.tile([P, FT, N_TGT], BF16)
        xtg_d = xtgb.rearrange("(ft p) i -> ft p i", p=P)
        for ft in range(FT):
            for ih in range(2):
                nc.sync.dma_start(
                    xtg_sb[:, ft, ih * 512:(ih + 1) * 512],
                    xtg_d[ft][:, ih * 512:(ih + 1) * 512])

        q_sb = persist.tile([P, FT, N_TGT], F8)
        e_sb = persist.tile([P, JT, N_TGT], F8)         # 8 MB resident
        colsum_sb = persist.tile([P, JT], F32)
        csg_sb = persist.tile([P, JT], F32)
        recip2_sb = persist.tile([P, JT], F32)
        ebias_sb = persist.tile([P, 1], F32)
        nc.vector.memset(ebias_sb[:], EXP_BIAS)


        # ---- phase V: v_src own shard [1024, H] bf16 -> AllGather ----
        # bias folded in as a ones-row matmul; psum evacuated on ScalarE
        vstores = []
        with (
            tc.tile_pool(name="psvq", bufs=4, space="PSUM") as psvq,
            tc.tile_pool(name="vt", bufs=3) as vtp,
        ):
            cc_vin_t = cc_vin.rearrange("(j4 p) f -> j4 p f", p=P)
            for j4 in range(N_SRCO // P):
                pv = psvq.tile([P, H], F32)
                for fs in range(FT):
                    nc.tensor.matmul(
                        pv[:],
                        xso_sb[:, fs, j4 * P:(j4 + 1) * P],
                        wvb_sb[:, fs, :],
                        start=(fs == 0), stop=(fs == FT - 1),
                    )
                vt = vtp.tile([P, H], BF16)
                nc.vector.tensor_add(vt[:], pv[:], bvb_sb[:])
                d = nc.sync.dma_start(cc_vin_t[j4], vt[:])
                vstores.append(d)
            ag = nc.gpsimd.collective_compute(
                "AllGather", ALU.bypass, replica_groups=rg,
                ins=[cc_vin[:]], outs=[cc_vout[:]],
            )
            for d in vstores:
                add_dep_helper(ag.ins, d.ins, sync=True,
                               reason="v_src stores before allgather")

            # ---- phase Q: q_ext^T = W2^T x_tgt + b2, cast to fp8 ----
            for ft in range(FT):
                for ic in range(2):
                    pq = psvq.tile([P, 512], F32)
                    for fs in range(FT):
                        nc.tensor.matmul(
                            pq[:],
                            w2b_sb[:, fs, ft * P:(ft + 1) * P],
                            xtg_sb[:, fs, ic * 512:(ic + 1) * 512],
                            start=(fs == 0), stop=(fs == FT - 1),
                        )
                    nc.scalar.activation(
                        q_sb[:, ft, ic * 512:(ic + 1) * 512], pq[:],
                        AF.Identity, bias=b2_sb[:, ft:ft + 1],
                    )

        # ---- A/B loop: fp8 DoubleRow scores, exp (fp8), g-weighted colsum ----
        xop = ctx.enter_context(tc.tile_pool(name="xo", bufs=3))
        xo_tiles = []
        oc_n = N_OWN // 512  # 6 chunks for phase E
        xs8_d = xs8.rearrange("(ft p) j -> ft p j", p=P)
        xo_d = xT_own.rearrange("(ft p) o -> ft p o", p=P)

        with (
            tc.tile_pool(name="xs", bufs=3) as xsp,
            tc.tile_pool(name="scr", bufs=2) as scrp,
            tc.tile_pool(name="pss", bufs=2, space="PSUM") as pss,
        ):
            for jb in range(NJB):
                xs = xsp.tile([P, FT, JBLK], F8, tag="xs")
                for ft in range(FT):
                    nc.sync.dma_start(
                        xs[:, ft, :],
                        xs8_d[ft][:, jb * JBLK:(jb + 1) * JBLK])
                for jp in range(JBLK // (2 * P)):   # jt pairs
                    jt0 = jb * (JBLK // P) + 2 * jp
                    ps = pss.tile([P, 2, N_TGT], F32)   # 4 PSUM banks
                    for h in range(2):
                        j4 = 2 * jp + h
                        for ic in range(2):
                            for k in range(2):
                                nc.tensor.matmul(
                                    ps[:, h, ic * 512:(ic + 1) * 512],
                                    xs[:, 2 * k:2 * k + 2,
                                       j4 * P:(j4 + 1) * P],
                                    q_sb[:, 2 * k:2 * k + 2,
                                         ic * 512:(ic + 1) * 512],
                                    start=(k == 0), stop=(k == 1),
                                    perf_mode=DR,
                                )
                    nc.scalar.activation(
                        e_sb[:, jt0:jt0 + 2, :], ps[:],
                        AF.Exp, scale=float(1.0 / np.sqrt(HEAD)),
                        bias=ebias_sb[:],
                    )
                    for h in range(2):
                        jt = jt0 + h
                        scr = scrp.tile([P, N_TGT], BF16, tag="scr")
                        nc.vector.scalar_tensor_tensor(
                            scr[:], e_sb[:, jt, :], 1.0, gbc_sb[:],
                            op0=ALU.bypass, op1=ALU.mult,
                            accum_out=colsum_sb[:, jt:jt + 1],
                        )
                # first-half colsum AllReduce, entirely on the gpsimd queue so
                # no engine instruction stream is ever blocked behind it
                if jb == NJB // 2 - 1:
                    d1a = nc.gpsimd.dma_start(cc_in_a[:],
                                              colsum_sb[:, :JT // 2])
                    ara = nc.gpsimd.collective_compute(
                        "AllReduce", ALU.add, replica_groups=rg,
                        ins=[cc_in_a[:]], outs=[cc_out_a[:]],
                    )
                    add_dep_helper(ara.ins, d1a.ins, sync=True,
                                   reason="colsum_a store before allreduce")
                    d2a = nc.gpsimd.dma_start(csg_sb[:, :JT // 2],
                                              cc_out_a[:])
                    add_dep_helper(d2a.ins, ara.ins, sync=True,
                                   reason="allreduce_a before readback")
                # recip of the first-half colsum: emitted at jb=13 so the DVE
                # reaches it well after AllReduce-a completes (never blocks)
                if jb == 13:
                    nc.vector.reciprocal(recip2_sb[:, :JT // 2],
                                         csg_sb[:, :JT // 2])
                    nc.vector.tensor_scalar_mul(
                        recip2_sb[:, :JT // 2], recip2_sb[:, :JT // 2], VSCALE)
                # prefetch first phase-E x chunks under the A/B loop
                if jb in (10, 12, 14):
                    oc = (jb - 10) // 2
                    xo = xop.tile([P, FT, 512], BF16, tag="xo")
                    for ft in range(FT):
                        nc.sync.dma_start(
                            xo[:, ft, :],
                            xo_d[ft][:, oc * 512:(oc + 1) * 512])
                    xo_tiles.append(xo)

        # ---- second-half colsum AllReduce (gpsimd queue) + both recips ----
        d1b = nc.gpsimd.dma_start(cc_in_b[:], colsum_sb[:, JT // 2:])
        arb = nc.gpsimd.collective_compute(
            "AllReduce", ALU.add, replica_groups=rg,
            ins=[cc_in_b[:]], outs=[cc_out_b[:]],
        )
        add_dep_helper(arb.ins, d1b.ins, sync=True,
                       reason="colsum_b store before allreduce")
        d2b = nc.gpsimd.dma_start(csg_sb[:, JT // 2:], cc_out_b[:])
        add_dep_helper(d2b.ins, arb.ins, sync=True,
                       reason="allreduce_b before readback")
        nc.vector.reciprocal(recip2_sb[:, JT // 2:], csg_sb[:, JT // 2:])
        nc.vector.tensor_scalar_mul(
            recip2_sb[:, JT // 2:], recip2_sb[:, JT // 2:], VSCALE)

        # ---- phase E: v_own = x_own @ Wv.T + bv (bf16; overlaps AllReduce) ----
        with (
            tc.tile_pool(name="vo", bufs=3) as vop,
            tc.tile_pool(name="pse", bufs=2, space="PSUM") as pse,
        ):
            v_own_t = v_own.rearrange("(ot p) f -> ot p f", p=P)
            for oc in range(oc_n):
                if oc < len(xo_tiles):
                    xo = xo_tiles[oc]
                else:
                    xo = xop.tile([P, FT, 512], BF16, tag="xo")
                    for ft in range(FT):
                        nc.sync.dma_start(
                            xo[:, ft, :],
                            xo_d[ft][:, oc * 512:(oc + 1) * 512])
                for o4 in range(4):
                    pe_ = pse.tile([P, H], F32)
                    for fs in range(FT):
                        nc.tensor.matmul(
                            pe_[:],
                            xo[:, fs, o4 * P:(o4 + 1) * P],
                            wvb_sb[:, fs, :],
                            start=(fs == 0), stop=(fs == FT - 1),
                        )
                    vo = vop.tile([P, H], BF16)
                    nc.scalar.copy(vo[:], pe_[:])  # bv added on host
                    nc.sync.dma_start(v_own_t[oc * 4 + o4], vo[:])

        # ---- phase C/D: out^T = e^T-matmul with (v*2048/colsum) in fp8 DR ----
        with (
            tc.tile_pool(name="cvb", bufs=10) as cvb,
            tc.tile_pool(name="cv8", bufs=3) as cv8,
            tc.tile_pool(name="co", bufs=2) as cop,
            tc.tile_pool(name="psc", bufs=1, space="PSUM") as pscp,
        ):
            psc_f = [pscp.tile([P, N_TGT], F32, tag=f"psc{f}", name=f"psc{f}")
                     for f in range(FT)]
            cc_vout_t = cc_vout.rearrange("(jt p) f -> jt p f", p=P)
            for t in range(JT // 2):
                v2 = cv8.tile([P, 2, H], F8, tag="v2")
                for k in range(2):
                    jt = 2 * t + k
                    vt = cvb.tile([P, H], BF16, tag="vt")
                    dv = nc.sync.dma_start(vt[:], cc_vout_t[jt])
                    add_dep_helper(dv.ins, ag.ins, sync=True,
                                   reason="allgather before v reload")
                    nc.scalar.mul(v2[:, k, :], vt[:],
                                  recip2_sb[:, jt:jt + 1])
                for ft in range(FT):
                    for ic in range(2):
                        nc.tensor.matmul(
                            psc_f[ft][:, ic * 512:(ic + 1) * 512],
                            v2[:, :, ft * P:(ft + 1) * P],
                            e_sb[:, 2 * t:2 * t + 2, ic * 512:(ic + 1) * 512],
                            start=(t == 0), stop=(t == JT // 2 - 1),
                            perf_mode=DR,
                        )
            for ft in range(FT):
                ot = cop.tile([P, N_TGT], F32)
                nc.vector.scalar_tensor_tensor(
                    ot[:], psc_f[ft][:], float(1.0 / VSCALE), gbc_sb[:],
                    op0=ALU.mult, op1=ALU.mult,
                )
                nc.sync.dma_start(outT_tgt[ft * P:(ft + 1) * P, :], ot[:])

    nc.compile()
    return nc


def _get_program():
    global _PROGRAM
    if _PROGRAM is None:
        _PROGRAM = build_program()
    return _PROGRAM


def make_in_maps(hidden_states, Wq, bq, Wk, bk, Wv, bv, edges_src, edges_tgt):
    """Host-side sharding: sort indices, gather rows, fold weights, cast."""
    import ml_dtypes
    BF = ml_dtypes.bfloat16
    F8NP = ml_dtypes.float8_e4m3

    x = np.ascontiguousarray(
        np.asarray(hidden_states, dtype=np.float32).reshape(N, H))
    src = np.sort(np.asarray(edges_src).astype(np.int64))
    tgt = np.sort(np.asarray(edges_tgt).astype(np.int64))
    mask = np.ones(N, bool)
    mask[tgt] = False
    nontgt = np.nonzero(mask)[0]
    xT = np.ascontiguousarray(x.T)                      # [H, N]

    # weight folding (f64 for exactness)
    Wq64 = np.asarray(Wq, np.float64)
    Wk64 = np.asarray(Wk, np.float64)
    bq64 = np.asarray(bq, np.float64)
    bk64 = np.asarray(bk, np.float64)
    W2 = (Wq64.T @ Wk64).astype(np.float32)
    b2 = (bq64 @ Wk64).astype(np.float32)
    wc = (Wq64.T @ bk64).astype(np.float32)
    beta = float(bq64 @ bk64)

    xT_src = xT[:, src]
    xs8 = np.ascontiguousarray(xT_src.astype(F8NP))
    xso_all = np.ascontiguousarray(xT_src.astype(BF))
    w2b = np.ascontiguousarray(W2.astype(BF))
    b2_t = np.ascontiguousarray(b2.reshape(FT, P).T)
    wvT = np.ascontiguousarray(np.asarray(Wv, np.float32).T)
    wvbT = np.ascontiguousarray(wvT.astype(BF))
    bv_bc = np.ascontiguousarray(
        np.tile(np.asarray(bv, np.float32)[None, :], (P, 1)))

    in_maps = []
    for c in range(N_CORES):
        tgt_c = tgt[c * N_TGT:(c + 1) * N_TGT]
        x_tgt_c = np.ascontiguousarray(xT[:, tgt_c])
        g = np.exp((x_tgt_c.T.astype(np.float64) @ wc.astype(np.float64)
                    + beta) / np.sqrt(HEAD)).astype(np.float32)
        in_maps.append({
            "xs8": xs8,
            "xso": np.ascontiguousarray(
                xso_all[:, c * N_SRCO:(c + 1) * N_SRCO]),
            "w2b": w2b,
            "xtgb": np.ascontiguousarray(x_tgt_c.astype(BF)),
            "b2_t": b2_t,
            "g_bc": np.ascontiguousarray(np.tile(g[None, :], (P, 1))),
            "wvbT": wvbT,
            "bv_bc": bv_bc,
            "xT_own": np.ascontiguousarray(
                xT[:, nontgt[c * N_OWN:(c + 1) * N_OWN]].astype(BF)),
        })
    return in_maps, tgt, nontgt


def assemble_output(results, tgt, nontgt, bv):
    v = np.empty((N, H), np.float32)
    bvf = np.asarray(bv, np.float32)[None, :]
    for c in range(N_CORES):
        v[nontgt[c * N_OWN:(c + 1) * N_OWN]] = \
            results[c]["v_own"].astype(np.float32) + bvf
        v[tgt[c * N_TGT:(c + 1) * N_TGT]] = results[c]["outT_tgt"].T
    return v.reshape(B, S, H)


def kernel(hidden_states, Wq, bq, Wk, bk, Wv, bv, edges_src, edges_tgt):
    global LAST_RESULT
    in_maps, tgt, nontgt = make_in_maps(
        hidden_states, Wq, bq, Wk, bk, Wv, bv, edges_src, edges_tgt)
    nc = _get_program()
    res = run_bass_kernel_spmd(nc, in_maps, list(range(N_CORES)))
    LAST_RESULT = res
    return assemble_output(res.results, tgt, nontgt, bv)
